# revision 45
# baseline (speedup 1.0000x reference)
"""GAT (2-layer, PyG-style) Trainium2 Bass kernel, 8-core SPMD.

Destination-node partitioning: each core owns a contiguous range of dst nodes
and all edges into it (host pre-groups edges by (dst block, src half)).

Layer 1 gathers raw x rows (padded to 256B, bf16) with dma_gather
transpose=True so they arrive channel-partitioned, then expands h|a_src per
edge with a K=16 matmul against W1|W1@As (no node table, no AllGather).
Layer 2 builds a per-node table [h2|as2] (768B bf16 rows) during layer-1
post-processing, AllGathers it once, and edge-gathers rows directly.

Per edge tile (128 edges): a_dst arrives via a small DT@av matmul (DT = host
one-hot dst transpose), e4 = exp(leakyrelu(as+ad)) is fused into the message
multiply, and a device-generated one-hot D (iota is_equal dloc) turns
scatter-add into PSUM matmul accumulation with softmax denominators riding as
4 extra rhs columns. Gathers round-robin over 4 SWDGE queues so descriptor
generation runs on all four Q7 cpu pairs concurrently.
"""

from contextlib import ExitStack

import numpy as np
import ml_dtypes

import concourse.bass as bass
import concourse.bacc as bacc
import concourse.mybir as mybir
import concourse.tile as tile
from concourse.masks import make_identity

P = 128
NC = 8
IN_CH = 16
HEADS = 4
HID = 64
C = HEADS * HID          # 256
OUT_CH = 8
ELEM2 = 384              # L2 table row: h(256) | as(4) | pad -> 384 bf16 = 768 B
NEG_SLOPE = 0.2
SB = 2                   # dst blocks per gather call
NQ = 4                   # SWDGE queues
F32 = mybir.dt.float32
BF16 = mybir.dt.bfloat16
I16 = mybir.dt.int16

BF1 = np.uint16(0x3F80)  # 1.0 in bf16 bits


def _bf16(x):
    return np.asarray(x, ml_dtypes.bfloat16).view(np.uint16)


# ----------------------------------------------------------------------------
# host-side preprocessing
# ----------------------------------------------------------------------------

def _prep_edges(src, dst, npc):
    """Group edges per core by (dst block, src half); build shared tile meta
    plus per-core index/dloc/DT arrays."""
    npad = NC * npc
    half = npad // 2
    nb = npc // P
    assert npc % P == 0 and half <= 32768

    core_of = dst // npc
    per_core = []
    counts = np.zeros((NC, nb, 2), np.int64)
    for k in range(NC):
        sel = core_of == k
        s = src[sel]
        dl = dst[sel] - k * npc
        blk = dl >> 7
        hlf = s // half
        order = np.lexsort((s, hlf, blk))
        s, dl, blk, hlf = s[order], dl[order], blk[order], hlf[order]
        np.add.at(counts[k], (blk, hlf), 1)
        # group start offsets in sorted arrays
        gstart = np.zeros((nb, 2), np.int64)
        gcnt = np.zeros((nb, 2), np.int64)
        idx = 0
        for b in range(nb):
            for h in range(2):
                cnt = int(((blk == b) & (hlf == h)).sum())
                gstart[b, h] = idx
                gcnt[b, h] = cnt
                idx += cnt
        per_core.append((s, dl, gstart, gcnt))

    T = np.ceil(counts.max(axis=0) / P).astype(np.int64)   # [nb, 2]
    TT = int(T.sum())
    tb = T.sum(axis=1)                                     # tiles per block
    tb_max = int(tb.max())

    nsb = -(-nb // SB)
    # calls: (sb, h) -> col start (in 16-wrapped units), nidx
    calls = []
    cs = 0
    call_id = {}
    for sb in range(nsb):
        blocks = list(range(sb * SB, min((sb + 1) * SB, nb)))
        for h in range(2):
            nidx = int(sum(T[b, h] for b in blocks) * P)
            call_id[(sb, h)] = len(calls)
            calls.append({"sb": sb, "h": h, "cs": cs, "nidx": nidx,
                          "blocks": blocks})
            cs += nidx // 16
    CT = cs

    # processing order tiles: for sb, for b in sb, for h, for tile
    tiles = []           # (b, h, call, off_in_call, pt)
    pt = 0
    pt_start = np.zeros(nb + 1, np.int64)
    for sb in range(nsb):
        blocks = calls[call_id[(sb, 0)]]["blocks"]
        for bi, b in enumerate(blocks):
            pt_start[b] = pt
            for h in range(2):
                off = int(sum(T[bb, h] for bb in blocks[:bi]))
                for i in range(int(T[b, h])):
                    tiles.append((b, h, call_id[(sb, h)], off + i, pt))
                    pt += 1
    pt_start[nb] = pt
    assert pt == TT

    meta = {"npc": npc, "npad": npad, "half": half, "nb": nb, "nsb": nsb,
            "T": T, "TT": TT, "tb": tb, "tb_max": tb_max, "calls": calls,
            "tiles": tiles, "pt_start": pt_start, "CT": CT}

    per_core_arrays = []
    for k in range(NC):
        s, dl, gstart, gcnt = per_core[k]
        idx16 = np.zeros((16, CT), np.int16)
        dloc = np.full(TT * P, -1, np.int64)
        gpos = np.zeros((nb, 2), np.int64)   # consumed edges per group
        for (b, h, c, off, ptt) in tiles:
            call = calls[c]
            g0 = int(gstart[b, h]) + int(gpos[b, h])
            n = min(int(gcnt[b, h]) - int(gpos[b, h]), P)
            gpos[b, h] += n
            if n <= 0:
                continue
            sl = np.arange(n)
            j = off * P + sl                      # slot within call
            col = call["cs"] + j // 16
            idx16[j % 16, col] = (s[g0:g0 + n] % half).astype(np.int16)
            dloc[ptt * P + sl] = dl[g0:g0 + n] & 127

        DT = np.zeros((P, TT * P), np.uint16)
        valid = dloc >= 0
        vs = np.where(valid)[0]
        DT[dloc[valid], vs] = BF1
        Dm = np.zeros((P, TT * P), np.uint16)
        Dm[vs % P, (vs // P) * P + dloc[valid]] = BF1

        per_core_arrays.append({
            "srcidx": np.tile(idx16, (8, 1)),
            "dlocc": np.ascontiguousarray(
                dloc.reshape(TT, P).T.astype(np.float32)),
            "DT": DT,
            "Dm": Dm,
        })
    return meta, per_core_arrays


def _fold_as(a_s):
    As = np.zeros((C, HEADS), np.float32)
    for h in range(HEADS):
        As[h * HID:(h + 1) * HID, h] = a_s[h]
    return As


# ----------------------------------------------------------------------------
# device program
# ----------------------------------------------------------------------------

def build_gat(tc, outs, ins, meta):
    nc = tc.nc
    npc, half, nb, nsb = meta["npc"], meta["half"], meta["nb"], meta["nsb"]
    npad = meta["npad"]
    T, calls, tiles = meta["T"], meta["calls"], meta["tiles"]
    tb, tb_max, TT = meta["tb"], meta["tb_max"], meta["TT"]
    pt_start = meta["pt_start"]
    phases = meta.get("phases", 4)

    t1_full = nc.dram_tensor("t1_full", [npad, ELEM2], BF16)
    t2_slice = nc.dram_tensor("t2_slice", [npc, ELEM2], BF16)
    t2_full = nc.dram_tensor("t2_full", [npad, ELEM2], BF16,
                             addr_space="Shared")
    dump = meta.get("dump")
    dbg = outs.get("dbg")

    with ExitStack() as ctx:
        consts = ctx.enter_context(tc.tile_pool(name="consts", bufs=1))
        gp = ctx.enter_context(tc.tile_pool(name="gp", bufs=4))
        dtp = ctx.enter_context(tc.tile_pool(name="dtp", bufs=2))
        dp = ctx.enter_context(tc.tile_pool(name="dp", bufs=2))
        mp = ctx.enter_context(tc.tile_pool(name="mp", bufs=2))
        zp = ctx.enter_context(tc.tile_pool(name="zp", bufs=2))
        e4p = ctx.enter_context(tc.tile_pool(name="e4p", bufs=2))
        pp = ctx.enter_context(tc.tile_pool(name="pp", bufs=1, space="PSUM"))

        # ---- constants
        idx_t = consts.tile([P, meta["CT"]], I16)
        nc.sync.dma_start(out=idx_t[:], in_=ins["srcidx"][:])
        w1_t = consts.tile([IN_CH, C + 8], BF16)
        nc.sync.dma_start(out=w1_t[:], in_=ins["W1avd"][:].bitcast(BF16))
        w2e_t = consts.tile([P, C + 8], BF16)
        nc.sync.dma_start(out=w2e_t[:], in_=ins["W2avdE"][:].bitcast(BF16))
        w2o_t = consts.tile([P, C + 8], BF16)
        nc.sync.dma_start(out=w2o_t[:], in_=ins["W2avdO"][:].bitcast(BF16))
        wce_t = consts.tile([HID // 2, OUT_CH], BF16)
        nc.sync.dma_start(out=wce_t[:], in_=ins["WcE"][:].bitcast(BF16))
        wco_t = consts.tile([HID // 2, OUT_CH], BF16)
        nc.sync.dma_start(out=wco_t[:], in_=ins["WcO"][:].bitcast(BF16))
        b1_t = consts.tile([P, C], F32)
        nc.sync.dma_start(out=b1_t[:], in_=ins["b1r"][:])
        b2_t = consts.tile([P, HID], F32)
        nc.sync.dma_start(out=b2_t[:], in_=ins["b2r"][:])
        bc_t = consts.tile([P, OUT_CH], F32)
        nc.sync.dma_start(out=bc_t[:], in_=ins["bcr"][:])
        ident = consts.tile([P, P], F32)
        make_identity(nc, ident[:])
        av1 = consts.tile([P, nb * HEADS], BF16)
        av2 = consts.tile([P, nb * HEADS], BF16)

        nregs = {}
        for c in calls:
            if c["nidx"] and c["nidx"] not in nregs:
                nregs[c["nidx"]] = nc.gpsimd.to_reg(c["nidx"])

        # ---- P-A: av1[n] = ad1 of local nodes; then build the FULL
        # layer-1 table t1 = [x@W1 | as1] locally, 4 blocks per iteration
        # (x is replicated across cores, so no collective is needed; h0
        # gathers start once the first node-half of the build lands).
        for b in range(nb):
            xTb = zp.tile([IN_CH, P], BF16, tag="xTb")
            nc.sync.dma_start(
                out=xTb[:], in_=ins["xTloc"][:, b * P:(b + 1) * P]
                    .bitcast(BF16))
            ps = pp.tile([P, C + 8], F32, tag="e", bufs=2)
            nc.tensor.matmul(ps[:], xTb[:], w1_t[:], start=True, stop=True)
            nc.scalar.activation(av1[:, b * HEADS:(b + 1) * HEADS],
                                 ps[:, C + 4:C + 8],
                                 mybir.ActivationFunctionType.Copy)
        Q4 = 4 * P
        for g4 in range(npad // Q4):
            xTg = zp.tile([IN_CH, Q4], BF16, tag="xTg")
            nc.sync.dma_start(
                out=xTg[:], in_=ins["xTfull"][:, g4 * Q4:(g4 + 1) * Q4]
                    .bitcast(BF16))
            st4 = zp.tile([P, 4, C + 4], BF16, tag="st4")
            for i in range(4):
                ps = pp.tile([P, C + 8], F32, tag="e", bufs=2)
                nc.tensor.matmul(ps[:], xTg[:, i * P:(i + 1) * P], w1_t[:],
                                 start=True, stop=True)
                nc.scalar.activation(st4[:, i, :], ps[:, 0:C + 4],
                                     mybir.ActivationFunctionType.Copy)
            nc.sync.dma_start(
                out=t1_full[g4 * Q4:(g4 + 1) * Q4, 0:C + 4]
                    .rearrange("(i p) c -> p i c", p=P),
                in_=st4[:])

        if phases < 2:
            return

        qrr = [0]

        def gather(c, layer):
            q = qrr[0] % NQ
            qrr[0] += 1
            nidx = c["nidx"]
            if nidx == 0:
                return None
            cs = c["cs"]
            h = c["h"]
            table = t1_full if layer == 1 else t2_full
            g = gp.tile([P, nidx // P, ELEM2], BF16, tag=f"g{h}", bufs=2)
            nc.gpsimd.dma_gather(
                out_ap=g[:], in_ap=table[h * half:(h + 1) * half, :],
                idxs_ap=idx_t[:, cs:cs + nidx // 16],
                num_idxs=nidx, num_idxs_reg=nregs[nidx],
                elem_size=ELEM2, transpose=False, single_packet=False,
                queue_num=q)
            return g

        def edge_pass(layer, post_fn):
            ti = 0
            for sb in range(nsb):
                c0 = calls[2 * sb]
                c1 = calls[2 * sb + 1]
                g0 = gather(c0, layer)
                g1 = gather(c1, layer)
                gs = (g0, g1)
                for bi, b in enumerate(c0["blocks"]):
                    ntile = int(tb[b])
                    if ntile == 0:
                        post_fn(b, None)
                        continue
                    p0 = int(pt_start[b])
                    t0 = int(T[b, 0])
                    t1 = int(T[b, 1])
                    off_base = [int(sum(T[bb, hh] for bb in
                                        c0["blocks"][:bi])) for hh in (0, 1)]
                    dt_t = dtp.tile([P, tb_max * P], BF16, tag="dt")
                    nc.sync.dma_start(
                        out=dt_t[:, 0:ntile * P],
                        in_=ins["DT"][:, p0 * P:(p0 + ntile) * P]
                            .bitcast(BF16))
                    d_blk = dp.tile([P, tb_max, P], BF16, tag="d")
                    nc.sync.dma_start(
                        out=d_blk[:, 0:ntile, :],
                        in_=ins["Dm"][:, p0 * P:(p0 + ntile) * P]
                            .bitcast(BF16))
                    m_blk = mp.tile([P, tb_max, C + 4], BF16, tag="m")
                    s4 = pp.tile([P, tb_max * HEADS], F32, tag="s4", bufs=2)
                    avs = (av1 if layer == 1 else av2)[
                        :, b * HEADS:(b + 1) * HEADS]
                    for lt in range(ntile):
                        bb, h, ci, off, ptt = tiles[ti]
                        assert bb == b and ptt == p0 + lt
                        ti += 1
                        g = gs[h]
                        r4 = s4[:, lt * HEADS:(lt + 1) * HEADS]
                        dts = dt_t[:, lt * P:(lt + 1) * P]
                        nc.tensor.matmul(r4, dts, avs, start=(lt == 0),
                                            stop=(lt == ntile - 1))
                    # ---- batched per-block e4 + fold
                    n4 = ntile * HEADS
                    s4sb = e4p.tile([P, tb_max * HEADS], F32, tag="s4sb")
                    if True:
                        for h, th, pos in ((0, t0, 0), (1, t1, t0)):
                            if th == 0:
                                continue
                            ob = off_base[h]
                            nc.vector.tensor_tensor(
                                out=s4sb[:, pos * HEADS:(pos + th) * HEADS]
                                    .rearrange("p (t j) -> p t j", j=HEADS),
                                in0=gs[h][:, ob:ob + th, C:C + 4],
                                in1=s4[:, pos * HEADS:(pos + th) * HEADS]
                                    .rearrange("p (t j) -> p t j", j=HEADS),
                                op=mybir.AluOpType.add)
                    s4v = s4sb[:, 0:n4]
                    t4b = e4p.tile([P, tb_max * HEADS], F32, tag="t4b")
                    nc.vector.tensor_scalar_mul(t4b[:, 0:n4], s4v, NEG_SLOPE)
                    t4a = e4p.tile([P, tb_max * HEADS], F32, tag="t4a")
                    nc.vector.tensor_tensor(
                        t4a[:, 0:n4], s4v, t4b[:, 0:n4],
                        op=mybir.AluOpType.max)
                    nc.scalar.activation(
                        m_blk[:, 0:ntile, C:C + 4],
                        t4a[:, 0:n4].rearrange("p (t j) -> p t j", j=HEADS),
                        mybir.ActivationFunctionType.Exp)
                    if True:
                        for h, th, pos in ((0, t0, 0), (1, t1, t0)):
                            if th == 0:
                                continue
                            ob = off_base[h]
                            nc.vector.tensor_tensor(
                                out=m_blk[:, pos:pos + th, 0:C]
                                    .rearrange("p t (h c) -> p t h c",
                                               h=HEADS),
                                in0=gs[h][:, ob:ob + th, 0:C]
                                    .rearrange("p t (h c) -> p t h c",
                                               h=HEADS),
                                in1=m_blk[:, pos:pos + th, C:C + 4]
                                    .unsqueeze(-1)
                                    .to_broadcast([P, th, HEADS, HID]),
                                op=mybir.AluOpType.mult)
                    if dump and dump.startswith("dm") and b == 0 \
                            and layer == 1:
                        dlt = int(dump[2:])
                        dt_dbg4 = zp.tile([P, P + C + 4], F32, tag="dbg4")
                        nc.vector.tensor_copy(dt_dbg4[:, 0:P], d_blk[:, dlt])
                        nc.vector.tensor_copy(dt_dbg4[:, P:P + C + 4],
                                              m_blk[:, dlt])
                        nc.sync.dma_start(out=dbg[:, 0:P + C + 4],
                                          in_=dt_dbg4[:])
                    # scatter-accumulate: one matmul chain into PSUM
                    psb = pp.tile([P, C + 4], F32, tag="blk", bufs=2)
                    for lt in range(ntile):
                        nc.tensor.matmul(
                            psb[:], d_blk[:, lt], m_blk[:, lt, 0:C + 4],
                            start=(lt == 0), stop=(lt == ntile - 1))
                    if dump == "psb" and b == 0 and layer == 1:
                        dt_dbg3 = zp.tile([P, C + 4], F32, tag="dbg3")
                        nc.vector.tensor_copy(dt_dbg3[:], psb[:])
                        nc.sync.dma_start(out=dbg[:, 0:C + 4], in_=dt_dbg3[:])
                    post_fn(b, psb)

        def evict(psum, width, tag):
            """PSUM -> SBUF f32 via the scalar engine."""
            t = zp.tile([P, width], F32, tag=tag)
            nc.scalar.activation(t[:], psum,
                                 mybir.ActivationFunctionType.Copy)
            return t

        def normalize(zsb, out_ap):
            """out = zsb[:, 0:C] / broadcast(max(zsb[:, C:C+4], eps))"""
            rden = e4p.tile([P, HEADS], F32, tag="rd")
            nc.vector.tensor_scalar_max(rden[:], zsb[:, C:C + 4], 1e-30)
            nc.vector.reciprocal(rden[:], rden[:])
            nc.vector.tensor_tensor(
                out=out_ap.rearrange("p (h c) -> p h c", h=HEADS),
                in0=zsb[:, 0:C].rearrange("p (h c) -> p h c", h=HEADS),
                in1=rden[:].unsqueeze(-1).to_broadcast([P, HEADS, HID]),
                op=mybir.AluOpType.mult)
            return rden

        def elu_to_bf16(z, out_bf, width, tag):
            """out_bf = elu(z) in bf16: relu(z) + exp(-relu(-z)) - 1."""
            u = zp.tile([P, width], F32, tag=tag + "u")
            nc.scalar.activation(u[:], z,
                                 mybir.ActivationFunctionType.Relu,
                                 scale=-1.0)
            nc.scalar.activation(u[:], u[:],
                                 mybir.ActivationFunctionType.Exp,
                                 scale=-1.0)
            v = zp.tile([P, width], F32, tag=tag + "v")
            nc.scalar.activation(v[:], z,
                                 mybir.ActivationFunctionType.Relu)
            nc.vector.tensor_tensor(v[:], v[:], u[:],
                                    op=mybir.AluOpType.add)
            nc.vector.tensor_scalar_add(out_bf, v[:], -1.0)

        def pair_transpose(zb_bf16_asf32, rows):
            """[128, rows] f32(bf16-pairs) -> psum [rows, 128] transposed."""
            pt = pp.tile([P, P], F32, tag="post")
            nc.tensor.transpose(pt[0:rows, :], zb_bf16_asf32, ident[:])
            return pt

        def post1(b, psb):
            zb = zp.tile([P, C], BF16, tag="zb1")
            if psb is None:
                nc.vector.memset(zb[:], 0.0)
            else:
                zsb = evict(psb[:], C + 4, "zs1")
                z = zp.tile([P, C], F32, tag="z1")
                normalize(zsb, z[:])
                nc.vector.tensor_tensor(z[:], z[:], b1_t[:],
                                        op=mybir.AluOpType.add)
                elu_to_bf16(z[:], zb[:], C, "e1")
            if dump == "z1" and b == 0:
                zdbg = zp.tile([P, C], F32, tag="zdbg")
                nc.vector.tensor_copy(zdbg[:], zb[:])
                nc.sync.dma_start(out=dbg[:, 0:C], in_=zdbg[:])
            ptp = pair_transpose(zb[:].bitcast(F32), P)
            zT4 = evict(ptp[:], P, "zT4")
            zT4b = zT4[:].bitcast(BF16).rearrange("p (n two) -> p n two", two=2)
            ps2 = pp.tile([P, C + 8], F32, tag="post")
            nc.tensor.matmul(ps2[:], zT4b[:, :, 0], w2e_t[:],
                             start=True, stop=False)
            nc.tensor.matmul(ps2[:], zT4b[:, :, 1], w2o_t[:],
                             start=False, stop=True)
            st2 = zp.tile([P, C + 4], BF16, tag="st2")
            nc.scalar.activation(st2[:], ps2[:, 0:C + 4],
                                 mybir.ActivationFunctionType.Copy)
            nc.scalar.activation(av2[:, b * HEADS:(b + 1) * HEADS],
                                 ps2[:, C + 4:C + 8],
                                 mybir.ActivationFunctionType.Copy)
            nc.sync.dma_start(out=t2_slice[b * P:(b + 1) * P, 0:C + 4],
                              in_=st2[:])

        def post2(b, psb):
            hb = zp.tile([P, HID], BF16, tag="hb2")
            if psb is None:
                nc.vector.memset(hb[:], 0.0)
            else:
                zsb = evict(psb[:], C + 4, "zs2")
                rden = e4p.tile([P, HEADS], F32, tag="rd")
                nc.vector.tensor_scalar_max(rden[:], zsb[:, C:C + 4], 1e-30)
                nc.vector.reciprocal(rden[:], rden[:])
                nc.vector.tensor_scalar_mul(rden[:], rden[:], 1.0 / HEADS)
                zn = zp.tile([P, C], F32, tag="z2n")
                nc.vector.tensor_tensor(
                    out=zn[:].rearrange("p (h c) -> p h c", h=HEADS),
                    in0=zsb[:, 0:C].rearrange("p (h c) -> p h c", h=HEADS),
                    in1=rden[:].unsqueeze(-1).to_broadcast([P, HEADS, HID]),
                    op=mybir.AluOpType.mult)
                hm = zp.tile([P, HID], F32, tag="hm")
                nc.vector.tensor_reduce(
                    out=hm[:],
                    in_=zn[:].rearrange("p (h c) -> p c h", h=HEADS),
                    axis=mybir.AxisListType.X, op=mybir.AluOpType.add)
                nc.vector.tensor_tensor(hm[:], hm[:], b2_t[:],
                                        op=mybir.AluOpType.add)
                elu_to_bf16(hm[:], hb[:], HID, "e2")
            ptp = pair_transpose(hb[:].bitcast(F32), HID // 2)
            zT2 = zp.tile([HID // 2, P], F32, tag="zT2")
            nc.scalar.activation(zT2[:], ptp[0:HID // 2, :],
                                 mybir.ActivationFunctionType.Copy)
            zT2b = zT2[:].bitcast(BF16).rearrange("p (n two) -> p n two", two=2)
            psy = pp.tile([P, OUT_CH], F32, tag="post")
            nc.tensor.matmul(psy[:], zT2b[:, :, 0], wce_t[:],
                             start=True, stop=False)
            nc.tensor.matmul(psy[:], zT2b[:, :, 1], wco_t[:],
                             start=False, stop=True)
            yt = zp.tile([P, OUT_CH], F32, tag="yt")
            nc.vector.tensor_tensor(yt[:], psy[:], bc_t[:],
                                    op=mybir.AluOpType.add)
            nc.sync.dma_start(out=outs["y"][b * P:(b + 1) * P, :], in_=yt[:])

        # ---- P-B: layer-1 edge pass (builds t2_slice and av2 in post1)
        edge_pass(1, post1)

        if phases < 3:
            return
        # ---- P-C: AllGather layer-2 table
        nc.gpsimd.collective_compute(
            "AllGather", mybir.AluOpType.bypass,
            replica_groups=[list(range(NC))],
            ins=[t2_slice[:]], outs=[t2_full[:]],
        )

        if phases < 4:
            return
        # ---- P-D: layer-2 edge pass
        edge_pass(2, post2)


# ----------------------------------------------------------------------------
# entry point
# ----------------------------------------------------------------------------

def _prepare(inputs, n_nodes, npc):
    ei = np.asarray(inputs["edge_index"])
    src = np.concatenate([ei[0], np.arange(n_nodes, dtype=ei.dtype)])
    src = src.astype(np.int64)
    dst = np.concatenate([ei[1], np.arange(n_nodes, dtype=ei.dtype)])
    dst = dst.astype(np.int64)
    meta, per_core = _prep_edges(src, dst, npc)
    npad = meta["npad"]

    x = np.asarray(inputs["x"], np.float32)
    xtab = np.zeros((npad, P), np.float32)
    xtab[:n_nodes, 0:IN_CH] = x
    xtab = _bf16(xtab)
    xT = np.zeros((IN_CH, npad), np.float32)
    xT[:, :n_nodes] = x.T
    xT = _bf16(xT)

    W1 = np.asarray(inputs["W1"], np.float32)
    W2 = np.asarray(inputs["W2"], np.float32)
    W1avd = _bf16(np.concatenate(
        [W1, W1 @ _fold_as(np.asarray(inputs["as1"], np.float32)),
         W1 @ _fold_as(np.asarray(inputs["ad1"], np.float32))], axis=1))
    W2avd = np.concatenate(
        [W2, W2 @ _fold_as(np.asarray(inputs["as2"], np.float32)),
         W2 @ _fold_as(np.asarray(inputs["ad2"], np.float32))], axis=1)
    W2avdE = _bf16(W2avd[0::2])
    W2avdO = _bf16(W2avd[1::2])
    Wc = np.asarray(inputs["Wc"], np.float32)
    b1r = np.tile(np.asarray(inputs["b1"], np.float32)[None, :], (P, 1))
    b2r = np.tile(np.asarray(inputs["b2"], np.float32)[None, :], (P, 1))
    bcr = np.tile(np.asarray(inputs["bc"], np.float32)[None, :], (P, 1))
    iota = np.tile(np.arange(P, dtype=np.float32)[None, :], (P, 1))

    in_maps = []
    for k in range(NC):
        m = {
            "xtab": xtab,
            "xTloc": np.ascontiguousarray(xT[:, k * npc:(k + 1) * npc]),
            "xTfull": xT,
            "W1avd": W1avd,
            "W2avdE": W2avdE, "W2avdO": W2avdO,
            "WcE": _bf16(Wc[0::2]), "WcO": _bf16(Wc[1::2]),
            "b1r": b1r, "b2r": b2r, "bcr": bcr, "iota": iota,
            "srcidx": per_core[k]["srcidx"],
            "dlocc": per_core[k]["dlocc"],
            "DT": per_core[k]["DT"],
            "Dm": per_core[k]["Dm"],
        }
        in_maps.append(m)
    return meta, in_maps


def _declare_and_build(nc, meta, sample_map):
    ins = {}
    for name, arr in sample_map.items():
        ins[name] = nc.dram_tensor(
            name, list(arr.shape), mybir.dt.from_np(arr.dtype),
            kind="ExternalInput").ap()
    y = nc.dram_tensor("y", [meta["npc"], OUT_CH], F32, kind="ExternalOutput")
    outs = {"y": y.ap()}
    if meta.get("dump"):
        dbg = nc.dram_tensor("dbg", [P, 512], F32, kind="ExternalOutput")
        outs["dbg"] = dbg.ap()
    with tile.TileContext(nc) as tc:
        build_gat(tc, outs, ins, meta)
    nc.compile()


TRACE = False
LAST_RESULT = None
PHASES = 4
DUMP = None
CORES = NC


def kernel(**inputs) -> np.ndarray:
    global LAST_RESULT
    from concourse.bass_utils import run_bass_kernel_spmd

    n_nodes = inputs["x"].shape[0]
    npc = -(-n_nodes // (NC * P)) * P
    meta, in_maps = _prepare(inputs, n_nodes, npc)
    meta["phases"] = PHASES
    meta["dump"] = DUMP

    nc = bacc.Bacc("TRN2", target_bir_lowering=False, num_swdge_queues=NQ)
    _declare_and_build(nc, meta, in_maps[0])

    res = run_bass_kernel_spmd(nc, in_maps[:CORES], core_ids=list(range(CORES)),
                               trace=TRACE)
    LAST_RESULT = res
    y = np.concatenate([r["y"] for r in res.results], axis=0)[:n_nodes]
    return y.astype(np.float32)


# revision 47
# speedup vs baseline: 1.0433x; 1.0433x over previous
"""GAT (2-layer, PyG-style) Trainium2 Bass kernel, 8-core SPMD.

Destination-node partitioning: each core owns a contiguous range of dst nodes
and all edges into it (host pre-groups edges by (dst block, src half)).

Layer 1 gathers raw x rows (padded to 256B, bf16) with dma_gather
transpose=True so they arrive channel-partitioned, then expands h|a_src per
edge with a K=16 matmul against W1|W1@As (no node table, no AllGather).
Layer 2 builds a per-node table [h2|as2] (768B bf16 rows) during layer-1
post-processing, AllGathers it once, and edge-gathers rows directly.

Per edge tile (128 edges): a_dst arrives via a small DT@av matmul (DT = host
one-hot dst transpose), e4 = exp(leakyrelu(as+ad)) is fused into the message
multiply, and a device-generated one-hot D (iota is_equal dloc) turns
scatter-add into PSUM matmul accumulation with softmax denominators riding as
4 extra rhs columns. Gathers round-robin over 4 SWDGE queues so descriptor
generation runs on all four Q7 cpu pairs concurrently.
"""

from contextlib import ExitStack

import numpy as np
import ml_dtypes

import concourse.bass as bass
import concourse.bacc as bacc
import concourse.mybir as mybir
import concourse.tile as tile
from concourse.masks import make_identity

P = 128
NC = 8
IN_CH = 16
HEADS = 4
HID = 64
C = HEADS * HID          # 256
OUT_CH = 8
ELEM2 = 384              # L2 table row: h(256) | as(4) | pad -> 384 bf16 = 768 B
NEG_SLOPE = 0.2
SB = 2                   # dst blocks per gather call
NQ = 4                   # SWDGE queues
F32 = mybir.dt.float32
BF16 = mybir.dt.bfloat16
I16 = mybir.dt.int16

BF1 = np.uint16(0x3F80)  # 1.0 in bf16 bits


def _bf16(x):
    return np.asarray(x, ml_dtypes.bfloat16).view(np.uint16)


# ----------------------------------------------------------------------------
# host-side preprocessing
# ----------------------------------------------------------------------------

def _prep_edges(src, dst, npc):
    """Group edges per core by (dst block, src half); build shared tile meta
    plus per-core index/dloc/DT arrays."""
    npad = NC * npc
    half = npad // 2
    nb = npc // P
    assert npc % P == 0 and half <= 32768

    core_of = dst // npc
    per_core = []
    counts = np.zeros((NC, nb, 2), np.int64)
    for k in range(NC):
        sel = core_of == k
        s = src[sel]
        dl = dst[sel] - k * npc
        blk = dl >> 7
        hlf = s // half
        order = np.lexsort((s, hlf, blk))
        s, dl, blk, hlf = s[order], dl[order], blk[order], hlf[order]
        np.add.at(counts[k], (blk, hlf), 1)
        # group start offsets in sorted arrays
        gstart = np.zeros((nb, 2), np.int64)
        gcnt = np.zeros((nb, 2), np.int64)
        idx = 0
        for b in range(nb):
            for h in range(2):
                cnt = int(((blk == b) & (hlf == h)).sum())
                gstart[b, h] = idx
                gcnt[b, h] = cnt
                idx += cnt
        per_core.append((s, dl, gstart, gcnt))

    T = np.ceil(counts.max(axis=0) / P).astype(np.int64)   # [nb, 2]
    TT = int(T.sum())
    tb = T.sum(axis=1)                                     # tiles per block
    tb_max = int(tb.max())

    nsb = -(-nb // SB)
    # calls: (sb, h) -> col start (in 16-wrapped units), nidx
    calls = []
    cs = 0
    call_id = {}
    for sb in range(nsb):
        blocks = list(range(sb * SB, min((sb + 1) * SB, nb)))
        for h in range(2):
            nidx = int(sum(T[b, h] for b in blocks) * P)
            call_id[(sb, h)] = len(calls)
            calls.append({"sb": sb, "h": h, "cs": cs, "nidx": nidx,
                          "blocks": blocks})
            cs += nidx // 16
    CT = cs

    # processing order tiles: for sb, for b in sb, for h, for tile
    tiles = []           # (b, h, call, off_in_call, pt)
    pt = 0
    pt_start = np.zeros(nb + 1, np.int64)
    for sb in range(nsb):
        blocks = calls[call_id[(sb, 0)]]["blocks"]
        for bi, b in enumerate(blocks):
            pt_start[b] = pt
            for h in range(2):
                off = int(sum(T[bb, h] for bb in blocks[:bi]))
                for i in range(int(T[b, h])):
                    tiles.append((b, h, call_id[(sb, h)], off + i, pt))
                    pt += 1
    pt_start[nb] = pt
    assert pt == TT

    meta = {"npc": npc, "npad": npad, "half": half, "nb": nb, "nsb": nsb,
            "T": T, "TT": TT, "tb": tb, "tb_max": tb_max, "calls": calls,
            "tiles": tiles, "pt_start": pt_start, "CT": CT}

    per_core_arrays = []
    for k in range(NC):
        s, dl, gstart, gcnt = per_core[k]
        idx16 = np.zeros((16, CT), np.int16)
        dloc = np.full(TT * P, -1, np.int64)
        gpos = np.zeros((nb, 2), np.int64)   # consumed edges per group
        for (b, h, c, off, ptt) in tiles:
            call = calls[c]
            g0 = int(gstart[b, h]) + int(gpos[b, h])
            n = min(int(gcnt[b, h]) - int(gpos[b, h]), P)
            gpos[b, h] += n
            if n <= 0:
                continue
            sl = np.arange(n)
            j = off * P + sl                      # slot within call
            col = call["cs"] + j // 16
            idx16[j % 16, col] = (s[g0:g0 + n] % half).astype(np.int16)
            dloc[ptt * P + sl] = dl[g0:g0 + n] & 127

        DT = np.zeros((P, TT * P), np.uint16)
        valid = dloc >= 0
        vs = np.where(valid)[0]
        DT[dloc[valid], vs] = BF1
        Dm = np.zeros((P, TT * P), np.uint16)
        Dm[vs % P, (vs // P) * P + dloc[valid]] = BF1

        per_core_arrays.append({
            "srcidx": np.tile(idx16, (8, 1)),
            "dlocc": np.ascontiguousarray(
                dloc.reshape(TT, P).T.astype(np.float32)),
            "DT": DT,
            "Dm": Dm,
        })
    return meta, per_core_arrays


def _fold_as(a_s):
    As = np.zeros((C, HEADS), np.float32)
    for h in range(HEADS):
        As[h * HID:(h + 1) * HID, h] = a_s[h]
    return As


# ----------------------------------------------------------------------------
# device program
# ----------------------------------------------------------------------------

def build_gat(tc, outs, ins, meta):
    nc = tc.nc
    npc, half, nb, nsb = meta["npc"], meta["half"], meta["nb"], meta["nsb"]
    npad = meta["npad"]
    T, calls, tiles = meta["T"], meta["calls"], meta["tiles"]
    tb, tb_max, TT = meta["tb"], meta["tb_max"], meta["TT"]
    pt_start = meta["pt_start"]
    phases = meta.get("phases", 4)

    t1_slice = nc.dram_tensor("t1_slice", [npc, ELEM2], BF16)
    t1_full = nc.dram_tensor("t1_full", [npad, ELEM2], BF16,
                             addr_space="Shared")
    t2_slice = nc.dram_tensor("t2_slice", [npc, ELEM2], BF16)
    t2_full = nc.dram_tensor("t2_full", [npad, ELEM2], BF16,
                             addr_space="Shared")
    dump = meta.get("dump")
    dbg = outs.get("dbg")

    with ExitStack() as ctx:
        consts = ctx.enter_context(tc.tile_pool(name="consts", bufs=1))
        gp = ctx.enter_context(tc.tile_pool(name="gp", bufs=4))
        dtp = ctx.enter_context(tc.tile_pool(name="dtp", bufs=2))
        dp = ctx.enter_context(tc.tile_pool(name="dp", bufs=2))
        mp = ctx.enter_context(tc.tile_pool(name="mp", bufs=2))
        zp = ctx.enter_context(tc.tile_pool(name="zp", bufs=2))
        e4p = ctx.enter_context(tc.tile_pool(name="e4p", bufs=2))
        pp = ctx.enter_context(tc.tile_pool(name="pp", bufs=1, space="PSUM"))

        # ---- constants
        idx_t = consts.tile([P, meta["CT"]], I16)
        nc.sync.dma_start(out=idx_t[:], in_=ins["srcidx"][:])
        w1_t = consts.tile([IN_CH, C + 8], BF16)
        nc.sync.dma_start(out=w1_t[:], in_=ins["W1avd"][:].bitcast(BF16))
        w2e_t = consts.tile([P, C + 8], BF16)
        nc.sync.dma_start(out=w2e_t[:], in_=ins["W2avdE"][:].bitcast(BF16))
        w2o_t = consts.tile([P, C + 8], BF16)
        nc.sync.dma_start(out=w2o_t[:], in_=ins["W2avdO"][:].bitcast(BF16))
        wce_t = consts.tile([HID // 2, OUT_CH], BF16)
        nc.sync.dma_start(out=wce_t[:], in_=ins["WcE"][:].bitcast(BF16))
        wco_t = consts.tile([HID // 2, OUT_CH], BF16)
        nc.sync.dma_start(out=wco_t[:], in_=ins["WcO"][:].bitcast(BF16))
        b1_t = consts.tile([P, C], F32)
        nc.sync.dma_start(out=b1_t[:], in_=ins["b1r"][:])
        b2_t = consts.tile([P, HID], F32)
        nc.sync.dma_start(out=b2_t[:], in_=ins["b2r"][:])
        bc_t = consts.tile([P, OUT_CH], F32)
        nc.sync.dma_start(out=bc_t[:], in_=ins["bcr"][:])
        ident = consts.tile([P, P], F32)
        make_identity(nc, ident[:])
        av1 = consts.tile([P, nb * HEADS], BF16)
        av2 = consts.tile([P, nb * HEADS], BF16)

        nregs = {}
        for c in calls:
            if c["nidx"] and c["nidx"] not in nregs:
                nregs[c["nidx"]] = nc.gpsimd.to_reg(c["nidx"])

        # ---- P-A: layer-1 node table t1 = [x@W1 | as1] and av1 = ad1
        for b in range(nb):
            xTb = zp.tile([IN_CH, P], BF16, tag="xTb")
            nc.sync.dma_start(
                out=xTb[:], in_=ins["xTloc"][:, b * P:(b + 1) * P]
                    .bitcast(BF16))
            ps = pp.tile([P, C + 8], F32, tag="e", bufs=2)
            nc.tensor.matmul(ps[:], xTb[:], w1_t[:], start=True, stop=True)
            st1 = zp.tile([P, C + 4], BF16, tag="st1")
            nc.scalar.activation(st1[:], ps[:, 0:C + 4],
                                 mybir.ActivationFunctionType.Copy)
            nc.scalar.activation(av1[:, b * HEADS:(b + 1) * HEADS],
                                 ps[:, C + 4:C + 8],
                                 mybir.ActivationFunctionType.Copy)
            nc.sync.dma_start(out=t1_slice[b * P:(b + 1) * P, 0:C + 4],
                              in_=st1[:])
        nc.gpsimd.collective_compute(
            "AllGather", mybir.AluOpType.bypass,
            replica_groups=[list(range(NC))],
            ins=[t1_slice[:]], outs=[t1_full[:]],
        )

        if phases < 2:
            return

        qrr = [0]

        def gather(c, layer):
            q = qrr[0] % NQ
            qrr[0] += 1
            nidx = c["nidx"]
            if nidx == 0:
                return None
            cs = c["cs"]
            h = c["h"]
            table = t1_full if layer == 1 else t2_full
            g = gp.tile([P, nidx // P, ELEM2], BF16, tag=f"g{h}", bufs=2)
            nc.gpsimd.dma_gather(
                out_ap=g[:], in_ap=table[h * half:(h + 1) * half, :],
                idxs_ap=idx_t[:, cs:cs + nidx // 16],
                num_idxs=nidx, num_idxs_reg=nregs[nidx],
                elem_size=ELEM2, transpose=False, single_packet=False,
                queue_num=q)
            return g

        def edge_pass(layer, post_fn):
            ti = 0
            for sb in range(nsb):
                c0 = calls[2 * sb]
                c1 = calls[2 * sb + 1]
                g0 = gather(c0, layer)
                g1 = gather(c1, layer)
                gs = (g0, g1)
                for bi, b in enumerate(c0["blocks"]):
                    ntile = int(tb[b])
                    if ntile == 0:
                        post_fn(b, None)
                        continue
                    p0 = int(pt_start[b])
                    t0 = int(T[b, 0])
                    t1 = int(T[b, 1])
                    off_base = [int(sum(T[bb, hh] for bb in
                                        c0["blocks"][:bi])) for hh in (0, 1)]
                    dt_t = dtp.tile([P, tb_max * P], BF16, tag="dt")
                    nc.sync.dma_start(
                        out=dt_t[:, 0:ntile * P],
                        in_=ins["DT"][:, p0 * P:(p0 + ntile) * P]
                            .bitcast(BF16))
                    d_blk = dp.tile([P, tb_max, P], BF16, tag="d")
                    nc.sync.dma_start(
                        out=d_blk[:, 0:ntile, :],
                        in_=ins["Dm"][:, p0 * P:(p0 + ntile) * P]
                            .bitcast(BF16))
                    m_blk = mp.tile([P, tb_max, C + 4], BF16, tag="m")
                    s4 = pp.tile([P, tb_max * HEADS], F32, tag="s4", bufs=2)
                    avs = (av1 if layer == 1 else av2)[
                        :, b * HEADS:(b + 1) * HEADS]
                    for lt in range(ntile):
                        bb, h, ci, off, ptt = tiles[ti]
                        assert bb == b and ptt == p0 + lt
                        ti += 1
                        g = gs[h]
                        r4 = s4[:, lt * HEADS:(lt + 1) * HEADS]
                        dts = dt_t[:, lt * P:(lt + 1) * P]
                        nc.tensor.matmul(r4, dts, avs, start=(lt == 0),
                                            stop=(lt == ntile - 1))
                    # ---- batched per-block e4 + fold
                    n4 = ntile * HEADS
                    s4sb = e4p.tile([P, tb_max * HEADS], F32, tag="s4sb")
                    if True:
                        for h, th, pos in ((0, t0, 0), (1, t1, t0)):
                            if th == 0:
                                continue
                            ob = off_base[h]
                            nc.vector.tensor_tensor(
                                out=s4sb[:, pos * HEADS:(pos + th) * HEADS]
                                    .rearrange("p (t j) -> p t j", j=HEADS),
                                in0=gs[h][:, ob:ob + th, C:C + 4],
                                in1=s4[:, pos * HEADS:(pos + th) * HEADS]
                                    .rearrange("p (t j) -> p t j", j=HEADS),
                                op=mybir.AluOpType.add)
                    s4v = s4sb[:, 0:n4]
                    t4b = e4p.tile([P, tb_max * HEADS], F32, tag="t4b")
                    nc.scalar.activation(t4b[:, 0:n4], s4v,
                                         mybir.ActivationFunctionType.Copy,
                                         scale=NEG_SLOPE)
                    t4a = e4p.tile([P, tb_max * HEADS], F32, tag="t4a")
                    nc.vector.tensor_tensor(
                        t4a[:, 0:n4], s4v, t4b[:, 0:n4],
                        op=mybir.AluOpType.max)
                    nc.scalar.activation(
                        m_blk[:, 0:ntile, C:C + 4],
                        t4a[:, 0:n4].rearrange("p (t j) -> p t j", j=HEADS),
                        mybir.ActivationFunctionType.Exp)
                    if True:
                        for h, th, pos in ((0, t0, 0), (1, t1, t0)):
                            if th == 0:
                                continue
                            ob = off_base[h]
                            nc.vector.tensor_tensor(
                                out=m_blk[:, pos:pos + th, 0:C]
                                    .rearrange("p t (h c) -> p t h c",
                                               h=HEADS),
                                in0=gs[h][:, ob:ob + th, 0:C]
                                    .rearrange("p t (h c) -> p t h c",
                                               h=HEADS),
                                in1=m_blk[:, pos:pos + th, C:C + 4]
                                    .unsqueeze(-1)
                                    .to_broadcast([P, th, HEADS, HID]),
                                op=mybir.AluOpType.mult)
                    if dump and dump.startswith("dm") and b == 0 \
                            and layer == 1:
                        dlt = int(dump[2:])
                        dt_dbg4 = zp.tile([P, P + C + 4], F32, tag="dbg4")
                        nc.vector.tensor_copy(dt_dbg4[:, 0:P], d_blk[:, dlt])
                        nc.vector.tensor_copy(dt_dbg4[:, P:P + C + 4],
                                              m_blk[:, dlt])
                        nc.sync.dma_start(out=dbg[:, 0:P + C + 4],
                                          in_=dt_dbg4[:])
                    # scatter-accumulate: one matmul chain into PSUM
                    psb = pp.tile([P, C + 4], F32, tag="blk", bufs=2)
                    for lt in range(ntile):
                        nc.tensor.matmul(
                            psb[:], d_blk[:, lt], m_blk[:, lt, 0:C + 4],
                            start=(lt == 0), stop=(lt == ntile - 1))
                    if dump == "psb" and b == 0 and layer == 1:
                        dt_dbg3 = zp.tile([P, C + 4], F32, tag="dbg3")
                        nc.vector.tensor_copy(dt_dbg3[:], psb[:])
                        nc.sync.dma_start(out=dbg[:, 0:C + 4], in_=dt_dbg3[:])
                    post_fn(b, psb)

        def evict(psum, width, tag):
            """PSUM -> SBUF f32 via the scalar engine."""
            t = zp.tile([P, width], F32, tag=tag)
            nc.scalar.activation(t[:], psum,
                                 mybir.ActivationFunctionType.Copy)
            return t

        def normalize(zsb, out_ap):
            """out = zsb[:, 0:C] / broadcast(max(zsb[:, C:C+4], eps))"""
            rden = e4p.tile([P, HEADS], F32, tag="rd")
            nc.vector.tensor_scalar_max(rden[:], zsb[:, C:C + 4], 1e-30)
            nc.vector.reciprocal(rden[:], rden[:])
            nc.vector.tensor_tensor(
                out=out_ap.rearrange("p (h c) -> p h c", h=HEADS),
                in0=zsb[:, 0:C].rearrange("p (h c) -> p h c", h=HEADS),
                in1=rden[:].unsqueeze(-1).to_broadcast([P, HEADS, HID]),
                op=mybir.AluOpType.mult)
            return rden

        def elu_to_bf16(z, out_bf, width, tag):
            """out_bf = elu(z) in bf16: relu(z) + exp(-relu(-z)) - 1."""
            u = zp.tile([P, width], F32, tag=tag + "u")
            nc.scalar.activation(u[:], z,
                                 mybir.ActivationFunctionType.Relu,
                                 scale=-1.0)
            nc.scalar.activation(u[:], u[:],
                                 mybir.ActivationFunctionType.Exp,
                                 scale=-1.0)
            v = zp.tile([P, width], F32, tag=tag + "v")
            nc.scalar.activation(v[:], z,
                                 mybir.ActivationFunctionType.Relu)
            nc.vector.tensor_tensor(v[:], v[:], u[:],
                                    op=mybir.AluOpType.add)
            nc.vector.tensor_scalar_add(out_bf, v[:], -1.0)

        def pair_transpose(zb_bf16_asf32, rows):
            """[128, rows] f32(bf16-pairs) -> psum [rows, 128] transposed."""
            pt = pp.tile([P, P], F32, tag="post")
            nc.tensor.transpose(pt[0:rows, :], zb_bf16_asf32, ident[:])
            return pt

        def post1(b, psb):
            zb = zp.tile([P, C], BF16, tag="zb1")
            if psb is None:
                nc.vector.memset(zb[:], 0.0)
            else:
                zsb = evict(psb[:], C + 4, "zs1")
                z = zp.tile([P, C], F32, tag="z1")
                normalize(zsb, z[:])
                nc.vector.tensor_tensor(z[:], z[:], b1_t[:],
                                        op=mybir.AluOpType.add)
                elu_to_bf16(z[:], zb[:], C, "e1")
            if dump == "z1" and b == 0:
                zdbg = zp.tile([P, C], F32, tag="zdbg")
                nc.vector.tensor_copy(zdbg[:], zb[:])
                nc.sync.dma_start(out=dbg[:, 0:C], in_=zdbg[:])
            ptp = pair_transpose(zb[:].bitcast(F32), P)
            zT4 = evict(ptp[:], P, "zT4")
            zT4b = zT4[:].bitcast(BF16).rearrange("p (n two) -> p n two", two=2)
            ps2 = pp.tile([P, C + 8], F32, tag="post")
            nc.tensor.matmul(ps2[:], zT4b[:, :, 0], w2e_t[:],
                             start=True, stop=False)
            nc.tensor.matmul(ps2[:], zT4b[:, :, 1], w2o_t[:],
                             start=False, stop=True)
            st2 = zp.tile([P, C + 4], BF16, tag="st2")
            nc.scalar.activation(st2[:], ps2[:, 0:C + 4],
                                 mybir.ActivationFunctionType.Copy)
            nc.scalar.activation(av2[:, b * HEADS:(b + 1) * HEADS],
                                 ps2[:, C + 4:C + 8],
                                 mybir.ActivationFunctionType.Copy)
            nc.sync.dma_start(out=t2_slice[b * P:(b + 1) * P, 0:C + 4],
                              in_=st2[:])

        def post2(b, psb):
            hb = zp.tile([P, HID], BF16, tag="hb2")
            if psb is None:
                nc.vector.memset(hb[:], 0.0)
            else:
                zsb = evict(psb[:], C + 4, "zs2")
                rden = e4p.tile([P, HEADS], F32, tag="rd")
                nc.vector.tensor_scalar_max(rden[:], zsb[:, C:C + 4], 1e-30)
                nc.vector.reciprocal(rden[:], rden[:])
                nc.vector.tensor_scalar_mul(rden[:], rden[:], 1.0 / HEADS)
                zn = zp.tile([P, C], F32, tag="z2n")
                nc.vector.tensor_tensor(
                    out=zn[:].rearrange("p (h c) -> p h c", h=HEADS),
                    in0=zsb[:, 0:C].rearrange("p (h c) -> p h c", h=HEADS),
                    in1=rden[:].unsqueeze(-1).to_broadcast([P, HEADS, HID]),
                    op=mybir.AluOpType.mult)
                hm = zp.tile([P, HID], F32, tag="hm")
                nc.vector.tensor_reduce(
                    out=hm[:],
                    in_=zn[:].rearrange("p (h c) -> p c h", h=HEADS),
                    axis=mybir.AxisListType.X, op=mybir.AluOpType.add)
                nc.vector.tensor_tensor(hm[:], hm[:], b2_t[:],
                                        op=mybir.AluOpType.add)
                elu_to_bf16(hm[:], hb[:], HID, "e2")
            ptp = pair_transpose(hb[:].bitcast(F32), HID // 2)
            zT2 = zp.tile([HID // 2, P], F32, tag="zT2")
            nc.scalar.activation(zT2[:], ptp[0:HID // 2, :],
                                 mybir.ActivationFunctionType.Copy)
            zT2b = zT2[:].bitcast(BF16).rearrange("p (n two) -> p n two", two=2)
            psy = pp.tile([P, OUT_CH], F32, tag="post")
            nc.tensor.matmul(psy[:], zT2b[:, :, 0], wce_t[:],
                             start=True, stop=False)
            nc.tensor.matmul(psy[:], zT2b[:, :, 1], wco_t[:],
                             start=False, stop=True)
            yt = zp.tile([P, OUT_CH], F32, tag="yt")
            nc.vector.tensor_tensor(yt[:], psy[:], bc_t[:],
                                    op=mybir.AluOpType.add)
            nc.sync.dma_start(out=outs["y"][b * P:(b + 1) * P, :], in_=yt[:])

        # ---- P-B: layer-1 edge pass (builds t2_slice and av2 in post1)
        edge_pass(1, post1)

        if phases < 3:
            return
        # ---- P-C: AllGather layer-2 table
        nc.gpsimd.collective_compute(
            "AllGather", mybir.AluOpType.bypass,
            replica_groups=[list(range(NC))],
            ins=[t2_slice[:]], outs=[t2_full[:]],
        )

        if phases < 4:
            return
        # ---- P-D: layer-2 edge pass
        edge_pass(2, post2)


# ----------------------------------------------------------------------------
# entry point
# ----------------------------------------------------------------------------

def _prepare(inputs, n_nodes, npc):
    ei = np.asarray(inputs["edge_index"])
    src = np.concatenate([ei[0], np.arange(n_nodes, dtype=ei.dtype)])
    src = src.astype(np.int64)
    dst = np.concatenate([ei[1], np.arange(n_nodes, dtype=ei.dtype)])
    dst = dst.astype(np.int64)
    meta, per_core = _prep_edges(src, dst, npc)
    npad = meta["npad"]

    x = np.asarray(inputs["x"], np.float32)
    xtab = np.zeros((npad, P), np.float32)
    xtab[:n_nodes, 0:IN_CH] = x
    xtab = _bf16(xtab)
    xT = np.zeros((IN_CH, npad), np.float32)
    xT[:, :n_nodes] = x.T
    xT = _bf16(xT)

    W1 = np.asarray(inputs["W1"], np.float32)
    W2 = np.asarray(inputs["W2"], np.float32)
    W1avd = _bf16(np.concatenate(
        [W1, W1 @ _fold_as(np.asarray(inputs["as1"], np.float32)),
         W1 @ _fold_as(np.asarray(inputs["ad1"], np.float32))], axis=1))
    W2avd = np.concatenate(
        [W2, W2 @ _fold_as(np.asarray(inputs["as2"], np.float32)),
         W2 @ _fold_as(np.asarray(inputs["ad2"], np.float32))], axis=1)
    W2avdE = _bf16(W2avd[0::2])
    W2avdO = _bf16(W2avd[1::2])
    Wc = np.asarray(inputs["Wc"], np.float32)
    b1r = np.tile(np.asarray(inputs["b1"], np.float32)[None, :], (P, 1))
    b2r = np.tile(np.asarray(inputs["b2"], np.float32)[None, :], (P, 1))
    bcr = np.tile(np.asarray(inputs["bc"], np.float32)[None, :], (P, 1))
    iota = np.tile(np.arange(P, dtype=np.float32)[None, :], (P, 1))

    in_maps = []
    for k in range(NC):
        m = {
            "xtab": xtab,
            "xTloc": np.ascontiguousarray(xT[:, k * npc:(k + 1) * npc]),
            "W1avd": W1avd,
            "W2avdE": W2avdE, "W2avdO": W2avdO,
            "WcE": _bf16(Wc[0::2]), "WcO": _bf16(Wc[1::2]),
            "b1r": b1r, "b2r": b2r, "bcr": bcr, "iota": iota,
            "srcidx": per_core[k]["srcidx"],
            "dlocc": per_core[k]["dlocc"],
            "DT": per_core[k]["DT"],
            "Dm": per_core[k]["Dm"],
        }
        in_maps.append(m)
    return meta, in_maps


def _declare_and_build(nc, meta, sample_map):
    ins = {}
    for name, arr in sample_map.items():
        ins[name] = nc.dram_tensor(
            name, list(arr.shape), mybir.dt.from_np(arr.dtype),
            kind="ExternalInput").ap()
    y = nc.dram_tensor("y", [meta["npc"], OUT_CH], F32, kind="ExternalOutput")
    outs = {"y": y.ap()}
    if meta.get("dump"):
        dbg = nc.dram_tensor("dbg", [P, 512], F32, kind="ExternalOutput")
        outs["dbg"] = dbg.ap()
    with tile.TileContext(nc) as tc:
        build_gat(tc, outs, ins, meta)
    nc.compile()


TRACE = False
LAST_RESULT = None
PHASES = 4
DUMP = None
CORES = NC


def kernel(**inputs) -> np.ndarray:
    global LAST_RESULT
    from concourse.bass_utils import run_bass_kernel_spmd

    n_nodes = inputs["x"].shape[0]
    npc = -(-n_nodes // (NC * P)) * P
    meta, in_maps = _prepare(inputs, n_nodes, npc)
    meta["phases"] = PHASES
    meta["dump"] = DUMP

    nc = bacc.Bacc("TRN2", target_bir_lowering=False, num_swdge_queues=NQ)
    _declare_and_build(nc, meta, in_maps[0])

    res = run_bass_kernel_spmd(nc, in_maps[:CORES], core_ids=list(range(CORES)),
                               trace=TRACE)
    LAST_RESULT = res
    y = np.concatenate([r["y"] for r in res.results], axis=0)[:n_nodes]
    return y.astype(np.float32)


# revision 49
# speedup vs baseline: 1.2174x; 1.1669x over previous
"""GAT (2-layer, PyG-style) Trainium2 Bass kernel, 8-core SPMD.

Destination-node partitioning: each core owns a contiguous range of dst nodes
and all edges into it (host pre-groups edges by (dst block, src half)).

Layer 1 gathers raw x rows (padded to 256B, bf16) with dma_gather
transpose=True so they arrive channel-partitioned, then expands h|a_src per
edge with a K=16 matmul against W1|W1@As (no node table, no AllGather).
Layer 2 builds a per-node table [h2|as2] (768B bf16 rows) during layer-1
post-processing, AllGathers it once, and edge-gathers rows directly.

Per edge tile (128 edges): a_dst arrives via a small DT@av matmul (DT = host
one-hot dst transpose), e4 = exp(leakyrelu(as+ad)) is fused into the message
multiply, and a device-generated one-hot D (iota is_equal dloc) turns
scatter-add into PSUM matmul accumulation with softmax denominators riding as
4 extra rhs columns. Gathers round-robin over 4 SWDGE queues so descriptor
generation runs on all four Q7 cpu pairs concurrently.
"""

from contextlib import ExitStack

import numpy as np
import ml_dtypes

import concourse.bass as bass
import concourse.bacc as bacc
import concourse.mybir as mybir
import concourse.tile as tile
from concourse.masks import make_identity

P = 128
NC = 8
IN_CH = 16
HEADS = 4
HID = 64
C = HEADS * HID          # 256
OUT_CH = 8
ELEM2 = 384              # L2 table row: h(256) | as(4) | pad -> 384 bf16 = 768 B
NEG_SLOPE = 0.2
SB = 2                   # dst blocks per gather call
NQ = 4                   # SWDGE queues
F32 = mybir.dt.float32
BF16 = mybir.dt.bfloat16
I16 = mybir.dt.int16

BF1 = np.uint16(0x3F80)  # 1.0 in bf16 bits


def _bf16(x):
    return np.asarray(x, ml_dtypes.bfloat16).view(np.uint16)


# ----------------------------------------------------------------------------
# host-side preprocessing
# ----------------------------------------------------------------------------

def _prep_edges(src, dst, npc):
    """Group edges per core by (dst block, src half); build shared tile meta
    plus per-core index/dloc/DT arrays."""
    npad = NC * npc
    half = npad // 2
    nb = npc // P
    assert npc % P == 0 and half <= 32768

    core_of = dst // npc
    per_core = []
    counts = np.zeros((NC, nb, 2), np.int64)
    for k in range(NC):
        sel = core_of == k
        s = src[sel]
        dl = dst[sel] - k * npc
        blk = dl >> 7
        hlf = s // half
        order = np.lexsort((s, hlf, blk))
        s, dl, blk, hlf = s[order], dl[order], blk[order], hlf[order]
        np.add.at(counts[k], (blk, hlf), 1)
        # group start offsets in sorted arrays
        gstart = np.zeros((nb, 2), np.int64)
        gcnt = np.zeros((nb, 2), np.int64)
        idx = 0
        for b in range(nb):
            for h in range(2):
                cnt = int(((blk == b) & (hlf == h)).sum())
                gstart[b, h] = idx
                gcnt[b, h] = cnt
                idx += cnt
        per_core.append((s, dl, gstart, gcnt))

    T = np.ceil(counts.max(axis=0) / P).astype(np.int64)   # [nb, 2]
    TT = int(T.sum())
    tb = T.sum(axis=1)                                     # tiles per block
    tb_max = int(tb.max())

    nsb = -(-nb // SB)
    # calls: (sb, h) -> col start (in 16-wrapped units), nidx
    calls = []
    cs = 0
    call_id = {}
    for sb in range(nsb):
        blocks = list(range(sb * SB, min((sb + 1) * SB, nb)))
        for h in range(2):
            nidx = int(sum(T[b, h] for b in blocks) * P)
            call_id[(sb, h)] = len(calls)
            calls.append({"sb": sb, "h": h, "cs": cs, "nidx": nidx,
                          "blocks": blocks})
            cs += nidx // 16
    CT = cs

    # processing order tiles: for sb, for b in sb, for h, for tile
    tiles = []           # (b, h, call, off_in_call, pt)
    pt = 0
    pt_start = np.zeros(nb + 1, np.int64)
    for sb in range(nsb):
        blocks = calls[call_id[(sb, 0)]]["blocks"]
        for bi, b in enumerate(blocks):
            pt_start[b] = pt
            for h in range(2):
                off = int(sum(T[bb, h] for bb in blocks[:bi]))
                for i in range(int(T[b, h])):
                    tiles.append((b, h, call_id[(sb, h)], off + i, pt))
                    pt += 1
    pt_start[nb] = pt
    assert pt == TT

    meta = {"npc": npc, "npad": npad, "half": half, "nb": nb, "nsb": nsb,
            "T": T, "TT": TT, "tb": tb, "tb_max": tb_max, "calls": calls,
            "tiles": tiles, "pt_start": pt_start, "CT": CT}

    per_core_arrays = []
    for k in range(NC):
        s, dl, gstart, gcnt = per_core[k]
        idx16 = np.zeros((16, CT), np.int16)
        dloc = np.full(TT * P, -1, np.int64)
        gpos = np.zeros((nb, 2), np.int64)   # consumed edges per group
        for (b, h, c, off, ptt) in tiles:
            call = calls[c]
            g0 = int(gstart[b, h]) + int(gpos[b, h])
            n = min(int(gcnt[b, h]) - int(gpos[b, h]), P)
            gpos[b, h] += n
            if n <= 0:
                continue
            sl = np.arange(n)
            j = off * P + sl                      # slot within call
            col = call["cs"] + j // 16
            idx16[j % 16, col] = (s[g0:g0 + n] % half).astype(np.int16)
            dloc[ptt * P + sl] = dl[g0:g0 + n] & 127

        DT = np.zeros((P, TT * P), np.uint16)
        valid = dloc >= 0
        vs = np.where(valid)[0]
        DT[dloc[valid], vs] = BF1
        Dm = np.zeros((P, TT * P), np.uint16)
        Dm[vs % P, (vs // P) * P + dloc[valid]] = BF1

        per_core_arrays.append({
            "srcidx": np.tile(idx16, (8, 1)),
            "dlocc": np.ascontiguousarray(
                dloc.reshape(TT, P).T.astype(np.float32)),
            "DT": DT,
            "Dm": Dm,
        })
    return meta, per_core_arrays


def _fold_as(a_s):
    As = np.zeros((C, HEADS), np.float32)
    for h in range(HEADS):
        As[h * HID:(h + 1) * HID, h] = a_s[h]
    return As


# ----------------------------------------------------------------------------
# device program
# ----------------------------------------------------------------------------

def build_gat(tc, outs, ins, meta):
    nc = tc.nc
    npc, half, nb, nsb = meta["npc"], meta["half"], meta["nb"], meta["nsb"]
    npad = meta["npad"]
    T, calls, tiles = meta["T"], meta["calls"], meta["tiles"]
    tb, tb_max, TT = meta["tb"], meta["tb_max"], meta["TT"]
    pt_start = meta["pt_start"]
    phases = meta.get("phases", 4)

    t1_slice = nc.dram_tensor("t1_slice", [npc, ELEM2], BF16)
    t1_full = nc.dram_tensor("t1_full", [npad, ELEM2], BF16,
                             addr_space="Shared")
    t2_slice = nc.dram_tensor("t2_slice", [npc, ELEM2], BF16)
    t2_full = nc.dram_tensor("t2_full", [npad, ELEM2], BF16,
                             addr_space="Shared")
    dump = meta.get("dump")
    dbg = outs.get("dbg")

    with ExitStack() as ctx:
        consts = ctx.enter_context(tc.tile_pool(name="consts", bufs=1))
        gp = ctx.enter_context(tc.tile_pool(name="gp", bufs=4))
        dtp = ctx.enter_context(tc.tile_pool(name="dtp", bufs=2))
        dp = ctx.enter_context(tc.tile_pool(name="dp", bufs=2))
        mp = ctx.enter_context(tc.tile_pool(name="mp", bufs=2))
        zp = ctx.enter_context(tc.tile_pool(name="zp", bufs=2))
        e4p = ctx.enter_context(tc.tile_pool(name="e4p", bufs=2))
        pp = ctx.enter_context(tc.tile_pool(name="pp", bufs=1, space="PSUM"))

        # ---- constants
        idx_t = consts.tile([P, meta["CT"]], I16)
        nc.sync.dma_start(out=idx_t[:], in_=ins["srcidx"][:])
        w1_t = consts.tile([IN_CH, C + 8], BF16)
        nc.sync.dma_start(out=w1_t[:], in_=ins["W1avd"][:].bitcast(BF16))
        w2e_t = consts.tile([P, C + 8], BF16)
        nc.sync.dma_start(out=w2e_t[:], in_=ins["W2avdE"][:].bitcast(BF16))
        w2o_t = consts.tile([P, C + 8], BF16)
        nc.sync.dma_start(out=w2o_t[:], in_=ins["W2avdO"][:].bitcast(BF16))
        wce_t = consts.tile([HID // 2, OUT_CH], BF16)
        nc.sync.dma_start(out=wce_t[:], in_=ins["WcE"][:].bitcast(BF16))
        wco_t = consts.tile([HID // 2, OUT_CH], BF16)
        nc.sync.dma_start(out=wco_t[:], in_=ins["WcO"][:].bitcast(BF16))
        b1_t = consts.tile([P, C], F32)
        nc.sync.dma_start(out=b1_t[:], in_=ins["b1r"][:])
        b2_t = consts.tile([P, HID], F32)
        nc.sync.dma_start(out=b2_t[:], in_=ins["b2r"][:])
        bc_t = consts.tile([P, OUT_CH], F32)
        nc.sync.dma_start(out=bc_t[:], in_=ins["bcr"][:])
        ident = consts.tile([P, P], F32)
        make_identity(nc, ident[:])
        av1 = consts.tile([P, nb * HEADS], BF16)
        av2 = consts.tile([P, nb * HEADS], BF16)

        nregs = {}
        for c in calls:
            if c["nidx"] and c["nidx"] not in nregs:
                nregs[c["nidx"]] = nc.gpsimd.to_reg(c["nidx"])

        # ---- P-A: layer-1 node table t1 = [x@W1 | as1] and av1 = ad1
        for b in range(nb):
            xTb = zp.tile([IN_CH, P], BF16, tag="xTb")
            nc.sync.dma_start(
                out=xTb[:], in_=ins["xTloc"][:, b * P:(b + 1) * P]
                    .bitcast(BF16))
            ps = pp.tile([P, C + 8], F32, tag="e", bufs=2)
            nc.tensor.matmul(ps[:], xTb[:], w1_t[:], start=True, stop=True)
            st1 = zp.tile([P, C + 4], BF16, tag="st1")
            nc.scalar.activation(st1[:], ps[:, 0:C + 4],
                                 mybir.ActivationFunctionType.Copy)
            nc.scalar.activation(av1[:, b * HEADS:(b + 1) * HEADS],
                                 ps[:, C + 4:C + 8],
                                 mybir.ActivationFunctionType.Copy)
            nc.sync.dma_start(out=t1_slice[b * P:(b + 1) * P, 0:C + 4],
                              in_=st1[:])
        nc.gpsimd.collective_compute(
            "AllGather", mybir.AluOpType.bypass,
            replica_groups=[list(range(NC))],
            ins=[t1_slice[:]], outs=[t1_full[:]],
        )

        if phases < 2:
            return

        qrr = [0]

        def gather(c, layer):
            q = qrr[0] % NQ
            qrr[0] += 1
            nidx = c["nidx"]
            if nidx == 0:
                return None
            cs = c["cs"]
            h = c["h"]
            table = t1_full if layer == 1 else t2_full
            g = gp.tile([P, nidx // P, ELEM2], BF16, tag=f"g{h}", bufs=3)
            nc.gpsimd.dma_gather(
                out_ap=g[:], in_ap=table[h * half:(h + 1) * half, :],
                idxs_ap=idx_t[:, cs:cs + nidx // 16],
                num_idxs=nidx, num_idxs_reg=nregs[nidx],
                elem_size=ELEM2, transpose=False, single_packet=False,
                queue_num=q)
            return g

        def edge_pass(layer, post_fn):
            ti = 0
            for sb in range(nsb):
                c0 = calls[2 * sb]
                c1 = calls[2 * sb + 1]
                g0 = gather(c0, layer)
                g1 = gather(c1, layer)
                gs = (g0, g1)
                for bi, b in enumerate(c0["blocks"]):
                    ntile = int(tb[b])
                    if ntile == 0:
                        post_fn(b, None)
                        continue
                    p0 = int(pt_start[b])
                    t0 = int(T[b, 0])
                    t1 = int(T[b, 1])
                    off_base = [int(sum(T[bb, hh] for bb in
                                        c0["blocks"][:bi])) for hh in (0, 1)]
                    dt_t = dtp.tile([P, tb_max * P], BF16, tag="dt")
                    nc.sync.dma_start(
                        out=dt_t[:, 0:ntile * P],
                        in_=ins["DT"][:, p0 * P:(p0 + ntile) * P]
                            .bitcast(BF16))
                    d_blk = dp.tile([P, tb_max, P], BF16, tag="d")
                    nc.sync.dma_start(
                        out=d_blk[:, 0:ntile, :],
                        in_=ins["Dm"][:, p0 * P:(p0 + ntile) * P]
                            .bitcast(BF16))
                    m_blk = mp.tile([P, tb_max, C + 4], BF16, tag="m")
                    s4 = pp.tile([P, tb_max * HEADS], F32, tag="s4", bufs=2)
                    avs = (av1 if layer == 1 else av2)[
                        :, b * HEADS:(b + 1) * HEADS]
                    for lt in range(ntile):
                        bb, h, ci, off, ptt = tiles[ti]
                        assert bb == b and ptt == p0 + lt
                        ti += 1
                        g = gs[h]
                        r4 = s4[:, lt * HEADS:(lt + 1) * HEADS]
                        dts = dt_t[:, lt * P:(lt + 1) * P]
                        nc.tensor.matmul(r4, dts, avs, start=(lt == 0),
                                            stop=(lt == ntile - 1))
                    # ---- batched per-block e4 + fold
                    n4 = ntile * HEADS
                    s4sb = e4p.tile([P, tb_max * HEADS], F32, tag="s4sb")
                    if True:
                        for h, th, pos in ((0, t0, 0), (1, t1, t0)):
                            if th == 0:
                                continue
                            ob = off_base[h]
                            nc.vector.tensor_tensor(
                                out=s4sb[:, pos * HEADS:(pos + th) * HEADS]
                                    .rearrange("p (t j) -> p t j", j=HEADS),
                                in0=gs[h][:, ob:ob + th, C:C + 4],
                                in1=s4[:, pos * HEADS:(pos + th) * HEADS]
                                    .rearrange("p (t j) -> p t j", j=HEADS),
                                op=mybir.AluOpType.add)
                    s4v = s4sb[:, 0:n4]
                    t4b = e4p.tile([P, tb_max * HEADS], F32, tag="t4b")
                    nc.vector.tensor_scalar_mul(t4b[:, 0:n4], s4v, NEG_SLOPE)
                    t4a = e4p.tile([P, tb_max * HEADS], F32, tag="t4a")
                    nc.vector.tensor_tensor(
                        t4a[:, 0:n4], s4v, t4b[:, 0:n4],
                        op=mybir.AluOpType.max)
                    nc.scalar.activation(
                        m_blk[:, 0:ntile, C:C + 4],
                        t4a[:, 0:n4].rearrange("p (t j) -> p t j", j=HEADS),
                        mybir.ActivationFunctionType.Exp)
                    if True:
                        for h, th, pos in ((0, t0, 0), (1, t1, t0)):
                            if th == 0:
                                continue
                            ob = off_base[h]
                            nc.vector.tensor_tensor(
                                out=m_blk[:, pos:pos + th, 0:C]
                                    .rearrange("p t (h c) -> p t h c",
                                               h=HEADS),
                                in0=gs[h][:, ob:ob + th, 0:C]
                                    .rearrange("p t (h c) -> p t h c",
                                               h=HEADS),
                                in1=m_blk[:, pos:pos + th, C:C + 4]
                                    .unsqueeze(-1)
                                    .to_broadcast([P, th, HEADS, HID]),
                                op=mybir.AluOpType.mult)
                    if dump and dump.startswith("dm") and b == 0 \
                            and layer == 1:
                        dlt = int(dump[2:])
                        dt_dbg4 = zp.tile([P, P + C + 4], F32, tag="dbg4")
                        nc.vector.tensor_copy(dt_dbg4[:, 0:P], d_blk[:, dlt])
                        nc.vector.tensor_copy(dt_dbg4[:, P:P + C + 4],
                                              m_blk[:, dlt])
                        nc.sync.dma_start(out=dbg[:, 0:P + C + 4],
                                          in_=dt_dbg4[:])
                    # scatter-accumulate: one matmul chain into PSUM
                    psb = pp.tile([P, C + 4], F32, tag="blk", bufs=2)
                    for lt in range(ntile):
                        nc.tensor.matmul(
                            psb[:], d_blk[:, lt], m_blk[:, lt, 0:C + 4],
                            start=(lt == 0), stop=(lt == ntile - 1))
                    if dump == "psb" and b == 0 and layer == 1:
                        dt_dbg3 = zp.tile([P, C + 4], F32, tag="dbg3")
                        nc.vector.tensor_copy(dt_dbg3[:], psb[:])
                        nc.sync.dma_start(out=dbg[:, 0:C + 4], in_=dt_dbg3[:])
                    post_fn(b, psb)

        def evict(psum, width, tag):
            """PSUM -> SBUF f32 via the scalar engine."""
            t = zp.tile([P, width], F32, tag=tag)
            nc.scalar.activation(t[:], psum,
                                 mybir.ActivationFunctionType.Copy)
            return t

        def normalize(zsb, out_ap):
            """out = zsb[:, 0:C] / broadcast(max(zsb[:, C:C+4], eps))"""
            rden = e4p.tile([P, HEADS], F32, tag="rd")
            nc.vector.tensor_scalar_max(rden[:], zsb[:, C:C + 4], 1e-30)
            nc.vector.reciprocal(rden[:], rden[:])
            nc.vector.tensor_tensor(
                out=out_ap.rearrange("p (h c) -> p h c", h=HEADS),
                in0=zsb[:, 0:C].rearrange("p (h c) -> p h c", h=HEADS),
                in1=rden[:].unsqueeze(-1).to_broadcast([P, HEADS, HID]),
                op=mybir.AluOpType.mult)
            return rden

        def elu_to_bf16(z, out_bf, width, tag):
            """out_bf = elu(z) in bf16: relu(z) + exp(-relu(-z)) - 1."""
            u = zp.tile([P, width], F32, tag=tag + "u")
            nc.scalar.activation(u[:], z,
                                 mybir.ActivationFunctionType.Relu,
                                 scale=-1.0)
            nc.scalar.activation(u[:], u[:],
                                 mybir.ActivationFunctionType.Exp,
                                 scale=-1.0)
            v = zp.tile([P, width], F32, tag=tag + "v")
            nc.scalar.activation(v[:], z,
                                 mybir.ActivationFunctionType.Relu)
            nc.vector.tensor_tensor(v[:], v[:], u[:],
                                    op=mybir.AluOpType.add)
            nc.vector.tensor_scalar_add(out_bf, v[:], -1.0)

        def pair_transpose(zb_bf16_asf32, rows):
            """[128, rows] f32(bf16-pairs) -> psum [rows, 128] transposed."""
            pt = pp.tile([P, P], F32, tag="post")
            nc.tensor.transpose(pt[0:rows, :], zb_bf16_asf32, ident[:])
            return pt

        def post1(b, psb):
            zb = zp.tile([P, C], BF16, tag="zb1")
            if psb is None:
                nc.vector.memset(zb[:], 0.0)
            else:
                zsb = evict(psb[:], C + 4, "zs1")
                z = zp.tile([P, C], F32, tag="z1")
                normalize(zsb, z[:])
                nc.vector.tensor_tensor(z[:], z[:], b1_t[:],
                                        op=mybir.AluOpType.add)
                elu_to_bf16(z[:], zb[:], C, "e1")
            if dump == "z1" and b == 0:
                zdbg = zp.tile([P, C], F32, tag="zdbg")
                nc.vector.tensor_copy(zdbg[:], zb[:])
                nc.sync.dma_start(out=dbg[:, 0:C], in_=zdbg[:])
            ptp = pair_transpose(zb[:].bitcast(F32), P)
            zT4 = evict(ptp[:], P, "zT4")
            zT4b = zT4[:].bitcast(BF16).rearrange("p (n two) -> p n two", two=2)
            ps2 = pp.tile([P, C + 8], F32, tag="post")
            nc.tensor.matmul(ps2[:], zT4b[:, :, 0], w2e_t[:],
                             start=True, stop=False)
            nc.tensor.matmul(ps2[:], zT4b[:, :, 1], w2o_t[:],
                             start=False, stop=True)
            st2 = zp.tile([P, C + 4], BF16, tag="st2")
            nc.scalar.activation(st2[:], ps2[:, 0:C + 4],
                                 mybir.ActivationFunctionType.Copy)
            nc.scalar.activation(av2[:, b * HEADS:(b + 1) * HEADS],
                                 ps2[:, C + 4:C + 8],
                                 mybir.ActivationFunctionType.Copy)
            nc.sync.dma_start(out=t2_slice[b * P:(b + 1) * P, 0:C + 4],
                              in_=st2[:])

        def post2(b, psb):
            hb = zp.tile([P, HID], BF16, tag="hb2")
            if psb is None:
                nc.vector.memset(hb[:], 0.0)
            else:
                zsb = evict(psb[:], C + 4, "zs2")
                rden = e4p.tile([P, HEADS], F32, tag="rd")
                nc.vector.tensor_scalar_max(rden[:], zsb[:, C:C + 4], 1e-30)
                nc.vector.reciprocal(rden[:], rden[:])
                nc.vector.tensor_scalar_mul(rden[:], rden[:], 1.0 / HEADS)
                zn = zp.tile([P, C], F32, tag="z2n")
                nc.vector.tensor_tensor(
                    out=zn[:].rearrange("p (h c) -> p h c", h=HEADS),
                    in0=zsb[:, 0:C].rearrange("p (h c) -> p h c", h=HEADS),
                    in1=rden[:].unsqueeze(-1).to_broadcast([P, HEADS, HID]),
                    op=mybir.AluOpType.mult)
                hm = zp.tile([P, HID], F32, tag="hm")
                nc.vector.tensor_reduce(
                    out=hm[:],
                    in_=zn[:].rearrange("p (h c) -> p c h", h=HEADS),
                    axis=mybir.AxisListType.X, op=mybir.AluOpType.add)
                nc.vector.tensor_tensor(hm[:], hm[:], b2_t[:],
                                        op=mybir.AluOpType.add)
                elu_to_bf16(hm[:], hb[:], HID, "e2")
            ptp = pair_transpose(hb[:].bitcast(F32), HID // 2)
            zT2 = zp.tile([HID // 2, P], F32, tag="zT2")
            nc.scalar.activation(zT2[:], ptp[0:HID // 2, :],
                                 mybir.ActivationFunctionType.Copy)
            zT2b = zT2[:].bitcast(BF16).rearrange("p (n two) -> p n two", two=2)
            psy = pp.tile([P, OUT_CH], F32, tag="post")
            nc.tensor.matmul(psy[:], zT2b[:, :, 0], wce_t[:],
                             start=True, stop=False)
            nc.tensor.matmul(psy[:], zT2b[:, :, 1], wco_t[:],
                             start=False, stop=True)
            yt = zp.tile([P, OUT_CH], F32, tag="yt")
            nc.vector.tensor_tensor(yt[:], psy[:], bc_t[:],
                                    op=mybir.AluOpType.add)
            nc.sync.dma_start(out=outs["y"][b * P:(b + 1) * P, :], in_=yt[:])

        # ---- P-B: layer-1 edge pass (builds t2_slice and av2 in post1)
        edge_pass(1, post1)

        if phases < 3:
            return
        # ---- P-C: AllGather layer-2 table
        nc.gpsimd.collective_compute(
            "AllGather", mybir.AluOpType.bypass,
            replica_groups=[list(range(NC))],
            ins=[t2_slice[:]], outs=[t2_full[:]],
        )

        if phases < 4:
            return
        # ---- P-D: layer-2 edge pass
        edge_pass(2, post2)


# ----------------------------------------------------------------------------
# entry point
# ----------------------------------------------------------------------------

def _prepare(inputs, n_nodes, npc):
    ei = np.asarray(inputs["edge_index"])
    src = np.concatenate([ei[0], np.arange(n_nodes, dtype=ei.dtype)])
    src = src.astype(np.int64)
    dst = np.concatenate([ei[1], np.arange(n_nodes, dtype=ei.dtype)])
    dst = dst.astype(np.int64)
    meta, per_core = _prep_edges(src, dst, npc)
    npad = meta["npad"]

    x = np.asarray(inputs["x"], np.float32)
    xtab = np.zeros((npad, P), np.float32)
    xtab[:n_nodes, 0:IN_CH] = x
    xtab = _bf16(xtab)
    xT = np.zeros((IN_CH, npad), np.float32)
    xT[:, :n_nodes] = x.T
    xT = _bf16(xT)

    W1 = np.asarray(inputs["W1"], np.float32)
    W2 = np.asarray(inputs["W2"], np.float32)
    W1avd = _bf16(np.concatenate(
        [W1, W1 @ _fold_as(np.asarray(inputs["as1"], np.float32)),
         W1 @ _fold_as(np.asarray(inputs["ad1"], np.float32))], axis=1))
    W2avd = np.concatenate(
        [W2, W2 @ _fold_as(np.asarray(inputs["as2"], np.float32)),
         W2 @ _fold_as(np.asarray(inputs["ad2"], np.float32))], axis=1)
    W2avdE = _bf16(W2avd[0::2])
    W2avdO = _bf16(W2avd[1::2])
    Wc = np.asarray(inputs["Wc"], np.float32)
    b1r = np.tile(np.asarray(inputs["b1"], np.float32)[None, :], (P, 1))
    b2r = np.tile(np.asarray(inputs["b2"], np.float32)[None, :], (P, 1))
    bcr = np.tile(np.asarray(inputs["bc"], np.float32)[None, :], (P, 1))
    iota = np.tile(np.arange(P, dtype=np.float32)[None, :], (P, 1))

    in_maps = []
    for k in range(NC):
        m = {
            "xtab": xtab,
            "xTloc": np.ascontiguousarray(xT[:, k * npc:(k + 1) * npc]),
            "W1avd": W1avd,
            "W2avdE": W2avdE, "W2avdO": W2avdO,
            "WcE": _bf16(Wc[0::2]), "WcO": _bf16(Wc[1::2]),
            "b1r": b1r, "b2r": b2r, "bcr": bcr, "iota": iota,
            "srcidx": per_core[k]["srcidx"],
            "dlocc": per_core[k]["dlocc"],
            "DT": per_core[k]["DT"],
            "Dm": per_core[k]["Dm"],
        }
        in_maps.append(m)
    return meta, in_maps


def _declare_and_build(nc, meta, sample_map):
    ins = {}
    for name, arr in sample_map.items():
        ins[name] = nc.dram_tensor(
            name, list(arr.shape), mybir.dt.from_np(arr.dtype),
            kind="ExternalInput").ap()
    y = nc.dram_tensor("y", [meta["npc"], OUT_CH], F32, kind="ExternalOutput")
    outs = {"y": y.ap()}
    if meta.get("dump"):
        dbg = nc.dram_tensor("dbg", [P, 512], F32, kind="ExternalOutput")
        outs["dbg"] = dbg.ap()
    with tile.TileContext(nc) as tc:
        build_gat(tc, outs, ins, meta)
    nc.compile()


TRACE = False
LAST_RESULT = None
PHASES = 4
DUMP = None
CORES = NC


def kernel(**inputs) -> np.ndarray:
    global LAST_RESULT
    from concourse.bass_utils import run_bass_kernel_spmd

    n_nodes = inputs["x"].shape[0]
    npc = -(-n_nodes // (NC * P)) * P
    meta, in_maps = _prepare(inputs, n_nodes, npc)
    meta["phases"] = PHASES
    meta["dump"] = DUMP

    nc = bacc.Bacc("TRN2", target_bir_lowering=False, num_swdge_queues=NQ)
    _declare_and_build(nc, meta, in_maps[0])

    res = run_bass_kernel_spmd(nc, in_maps[:CORES], core_ids=list(range(CORES)),
                               trace=TRACE)
    LAST_RESULT = res
    y = np.concatenate([r["y"] for r in res.results], axis=0)[:n_nodes]
    return y.astype(np.float32)
